# revision 24
# baseline (speedup 1.0000x reference)
"""Additive (Bahdanau) attention log-softmax weights on 8 TRN2 NeuronCores.

Math (per batch b, head 0):
    qp = Q @ Wq^T ; kp = K @ Wk^T          (Wc = [Wq | Wk], both [D, D])
    logit[q, k] = Wl . tanh(qp[q] + kp[k] + bc) + bl + where(mask[k]==0, -1e9, 1.0)
    out[q, :]   = log_softmax(logit[q, :])

Distribution: pure data parallel, core c <- (batch b = c//2, q-half c%2),
no collectives.  Sparse-attention trick: keys with mask==0 only need
out = -1e9 - LSE (error O(1) vs magnitude 1e9), so the device only computes
the ~136 valid keys (host compacts + pads to V).

Algorithm: polynomial separation instead of elementwise tanh.  With
A[e,q] = qp^T and K[e,k] = kp^T + bc, fit an odd minimax polynomial
p(x) = sum_j c_j x^j ~ tanh(x) on the exact realized range of A+K, then

    logit[q,k] ~ sum_e Wl[e] p(A+K) = sum_m <(Wl .* A^m)[:,q], R_m(K)[:,k]>

where R_m(K) = sum_l c_{m+l} C(m+l,m) K^l is elementwise in K.  The m=n
term is constant in k and cancels in log_softmax -> dropped.  This turns
8.9M tanh evaluations per core (58us on ACT at 1 col/cycle) into
 - a rank-(n*512) PE contraction (36 matmuls, ~4us),
 - Horner chains for R_m on DVE over only [512, V] elements (~12us),
 - powers A^m on DVE over [512, 128] (~3us),
and the logits land dense in PSUM [q, V], so softmax needs no gather.

Per-core layout (e' = partition, 4 e-chunks "ec" in the free dim):
  A, P_m : [128, 512]  col = ec*128 + q      (bf16)
  K, K2, R_m : [128, 4V]  col = ec*V + kc    (bf16)
  psL    : PSUM [128q, V] accumulating all 36 (m, ec) matmuls.
R_m chains are emitted step-round-robin (m descending) so R_8 finishes
first and PE starts accumulating while R_0 is still being built.

Timing note: the repeat-slope NEFF shares ONE output DRAM tensor across
repeats -- the axon tunnel charges ~80ms per output tensor (independent
of size/compute), which would otherwise dominate the slope.
"""

import numpy as np
import ml_dtypes
from contextlib import ExitStack

import concourse.bass as bass
import concourse.tile as tile
from concourse import bacc, mybir
from concourse.bass_utils import run_bass_kernel_spmd

F32 = mybir.dt.float32
BF16 = mybir.dt.bfloat16
AF = mybir.ActivationFunctionType
ALU = mybir.AluOpType

B, H, Lq, Lkv, D = 4, 1, 256, 256, 512
NCORES = 8
LQL = Lq // 2          # q rows per core
NEG = -1.0e9
NPOLY = 7              # odd minimax degree for tanh
ACT_HEAD_MIN = 4       # chains with m >= this run their head on ACT

_nc_cache: dict[int, object] = {}


def _chain_specs(n=None):
    """Per-m Horner chain structure for R_m(K) = sum_l beta_l K^l with
    m+l odd, l <= n-m.  Returns [(m, kfactor, d2)] with d2 = degree in K2
    of the inner polynomial; m even -> R_m = K * poly_d2(K2) (no constant),
    m odd -> R_m = poly_d2(K2) (with constant).  m = n dropped (k-constant
    term cancels in log_softmax)."""
    if n is None:
        n = NPOLY
    specs = []
    for m in range(n):
        lmax = n - m if (n - m + m) % 2 == 1 else n - m - 1
        # l of the same parity as (odd - m): l parity = (1 - m%2)
        if m % 2 == 0:
            d2 = (lmax - 1) // 2      # l = 1, 3, ..., lmax = 2*d2+1
            specs.append((m, True, d2))
        else:
            d2 = lmax // 2            # l = 0, 2, ..., lmax = 2*d2
            specs.append((m, False, d2))
    return specs


def _chain_coeffs(cj, n=None):
    """Flat coefficient list in the exact order _build consumes them.
    For chain m the Horner (over y=K2) coefficients are beta_{l(top)} ...
    beta_{l(bottom)}, where beta_l = c_{m+l} * C(m+l, m)."""
    from math import comb
    if n is None:
        n = NPOLY
    out = []
    for m, kfac, d2 in _chain_specs(n):
        if kfac:
            ls = [2 * i + 1 for i in range(d2, -1, -1)]
        else:
            ls = [2 * i for i in range(d2, -1, -1)]
        for l in ls:
            out.append(float(cj[m + l]) * comb(m + l, m))
    return out


def _build(V: int, repeats: int = 1, skip=(), pchain_dve=True):
    """Build + schedule the per-core Bass graph for padded-valid-count V.

    skip / pchain_dve are timing-ablation knobs (results become garbage for
    most of them); the real kernel uses the defaults."""
    nc = bacc.Bacc(None, target_bir_lowering=False)

    specs = _chain_specs()
    ncoef = sum(d2 + 1 for _, _, d2 in specs)

    # packed inputs: qkp = [qt | p0 | kt | (partition-0 rows: ones128, bc
    # chunks, onesV, bv)] bf16, aux = [wlp | coef] f32 -- each dma_start
    # costs ~0.6us of sequencer dispatch, so small tensors ride together.
    # The bc and mask biases are applied by tiny 1-partition PE matmuls
    # (rank-1 outer products with a ones row) instead of per-ec vector ops.
    W_QKP = 1664 + 6 * V
    p_qkp = nc.declare_dram_parameter("qkp", [128, W_QKP], BF16,
                                      isOutput=False)
    p_wct = nc.declare_dram_parameter("wct", [4, 128, 1024], BF16, isOutput=False)
    p_aux = nc.declare_dram_parameter("aux", [128, 4 + ncoef], F32,
                                      isOutput=False)
    # One output tensor shared by every repeat: the axon tunnel charges a
    # large fixed cost PER OUTPUT TENSOR, so the timing NEFF must not scale
    # its output count with R (WAW between repeats is queue-ordered).
    p_out = nc.declare_dram_parameter("out", [128, V + 1], F32, isOutput=True)

    with ExitStack() as ctx:
        tc = ctx.enter_context(tile.TileContext(nc))
        const = ctx.enter_context(tc.tile_pool(name="const", bufs=1))
        psum = ctx.enter_context(tc.tile_pool(name="psum", bufs=2, space="PSUM"))

        for rep in range(repeats):
            r = f"_r{rep}"
            # ---- loads (wct on the ACT queue, rest on SP queue) ----
            wct_t = [const.tile([128, 1024], BF16, tag=f"wct{ec}",
                                name=f"wct{ec}{r}") for ec in range(4)]
            if "loads" not in skip:
                for ec in range(4):
                    nc.sync.dma_start(wct_t[ec][:], p_wct[ec])
            qkp_t = const.tile([128, W_QKP], BF16, tag="qkp",
                               name=f"qkp{r}")
            if "loads" not in skip:
                nc.sync.dma_start(qkp_t[:], p_qkp[:])
            aux_t = const.tile([128, 4 + ncoef], F32, tag="aux",
                               name=f"aux{r}")
            if "loads" not in skip:
                nc.sync.dma_start(aux_t[:], p_aux[:])
            qt_t = qkp_t[:, 0:512]
            p0_t = qkp_t[:, 512:1024]
            kt_t = qkp_t[:, 1024:1024 + 4 * V]
            zb = 1024 + 4 * V
            ones128 = qkp_t[0:1, zb:zb + 128]
            bcrow = [qkp_t[0:1, zb + 128 + ec * 128:zb + 256 + ec * 128]
                     for ec in range(4)]
            onesV = qkp_t[0:1, zb + 640:zb + 640 + V]
            bvrow = qkp_t[0:1, zb + 640 + V:zb + 640 + 2 * V]
            wlp_t = aux_t[:, 0:4]
            coef_t = aux_t[:, 4:4 + ncoef]

            # ---- projections: qp^T -> A/P1, kp^T + bc -> K (folds on the
            #      otherwise-idle ACT engine; per-partition bias/scale APs) --
            K = const.tile([128, 4 * V], BF16, tag="K", name=f"K{r}")
            K2 = const.tile([128, 4 * V], BF16, tag="K2", name=f"K2{r}")
            A = const.tile([128, 512], BF16, tag="A", name=f"A{r}")
            P = [p0_t] + [const.tile([128, 512], BF16, tag=f"P{m}",
                                     name=f"P{m}{r}") for m in range(1, NPOLY)]
            R = [const.tile([128, 4 * V], BF16, tag=f"R{m}", name=f"R{m}{r}")
                 for m in range(NPOLY)]
            # k-projections first: the K -> K2 -> R chains are the critical
            # path; q-projections only feed the P side which has slack
            if "proj" in skip:
                nc.gpsimd.memset(K[:], 0.001)
                nc.gpsimd.memset(A[:], 0.001)
                nc.gpsimd.memset(P[1][:], 0.001)
            if "k2" in skip:
                nc.gpsimd.memset(K2[:], 0.001)
            if "loads" in skip:
                nc.gpsimd.memset(qkp_t[:], 0.001)
                nc.gpsimd.memset(aux_t[:], 0.001)
                for ec in range(4):
                    nc.gpsimd.memset(wct_t[ec][:], 0.001)
            if "proj" not in skip:
                # k-projection: two 2-bank PSUM tiles, per-ec accumulation
                # groups at disjoint columns; bc lands via a rank-1
                # (bc-chunk x onesV) matmul so the PSUM->SBUF copy is pure
                # and merges across ec pairs
                ps_kh = [psum.tile([128, 2 * V], F32, tag=f"psk{h}",
                                   name=f"psk{h}{r}") for h in range(2)]
                for ec in range(4):
                    dst = ps_kh[ec // 2][:, (ec % 2) * V:(ec % 2 + 1) * V]
                    for dc in range(4):
                        nc.tensor.matmul(
                            dst,
                            wct_t[ec][:, 512 + dc * 128:512 + (dc + 1) * 128],
                            kt_t[:, dc * V:(dc + 1) * V],
                            start=(dc == 0), stop=False)
                    nc.tensor.matmul(dst, bcrow[ec], onesV,
                                     start=False, stop=True)
                for h in range(2):
                    nc.scalar.activation(
                        K[:, h * 2 * V:(h + 1) * 2 * V], ps_kh[h][:], AF.Copy)
            if "k2" not in skip:
                nc.scalar.activation(K2[:], K[:], AF.Square)
            if "proj" not in skip:
                # q-projection: one single-bank [128, 512] PSUM tile, per-ec
                # groups at disjoint columns -> a single wide A copy
                ps_q = psum.tile([128, 512], F32, tag="psq", name=f"psq{r}")
                for ec in range(4):
                    for dc in range(4):
                        nc.tensor.matmul(
                            ps_q[:, ec * 128:(ec + 1) * 128],
                            wct_t[ec][:, dc * 128:(dc + 1) * 128],
                            qt_t[:, dc * 128:(dc + 1) * 128],
                            start=(dc == 0), stop=(dc == 3))
                nc.scalar.activation(A[:], ps_q[:], AF.Copy)
                for ec in range(4):
                    nc.vector.tensor_scalar_mul(
                        P[1][:, ec * 128:(ec + 1) * 128],
                        A[:, ec * 128:(ec + 1) * 128], wlp_t[:, ec:ec + 1])

            # R_m Horner chains over K2.  Chain heads (beta_top*K2 +
            # beta_next, a pure scale/bias op) for m >= ACT_HEAD_MIN run as
            # ACT Identity/Copy instructions with per-partition scale+bias
            # APs, balancing the two engines; the tensor*tensor inner mads
            # must stay on DVE.  DVE stream order: heads (m desc), then the
            # P-power chain (A is ready by then), then the inner mads
            # round-robin so high-m chains finish first for PE.
            cbase = {}
            idx = 0
            for m, kfac, d2 in specs:
                cbase[m] = idx
                idx += d2 + 1
            order = [s for s in reversed(specs)]   # m desc
            maxsteps = max(d2 for _, _, d2 in specs) + 2
            done = set()
            if "rchain" in skip:
                for m in range(NPOLY):
                    nc.gpsimd.memset(R[m][:], 0.001)

            def emit_head(m, kfac, d2):
                acc, c0 = R[m], cbase[m]
                if m >= ACT_HEAD_MIN:
                    if d2 == 0:
                        assert kfac
                        nc.scalar.activation(acc[:], K[:], AF.Copy,
                                             scale=coef_t[:, c0:c0 + 1])
                        done.add(m)
                    else:
                        nc.scalar.activation(acc[:], K2[:], AF.Identity,
                                             bias=coef_t[:, c0 + 1:c0 + 2],
                                             scale=coef_t[:, c0:c0 + 1])
                elif d2 == 0:
                    assert kfac
                    nc.vector.tensor_scalar_mul(acc[:], K[:],
                                                coef_t[:, c0:c0 + 1])
                    done.add(m)
                else:
                    # fused (K2 * beta_top + beta_next) in one 4x instr
                    nc.vector.tensor_scalar(
                        acc[:], K2[:], coef_t[:, c0:c0 + 1],
                        coef_t[:, c0 + 1:c0 + 2], op0=ALU.mult, op1=ALU.add)

            if "rchain" not in skip:
                for m, kfac, d2 in order:
                    emit_head(m, kfac, d2)

            if "pchain" in skip:
                for m in range(2, NPOLY):
                    nc.gpsimd.memset(P[m][:], 0.001)
            else:
                eng = nc.vector if pchain_dve else nc.gpsimd
                for m in range(2, NPOLY):
                    eng.tensor_tensor(P[m][:], P[m - 1][:], A[:], ALU.mult)

            for step in range(1 if "rchain" not in skip else 10**9,
                              maxsteps + 1):
                for m, kfac, d2 in order:
                    if m in done:
                        continue
                    acc = R[m]
                    c0 = cbase[m]
                    if step < d2:
                        nc.vector.tensor_tensor(acc[:], acc[:], K2[:], ALU.mult)
                        nc.vector.tensor_scalar_add(
                            acc[:], acc[:], coef_t[:, c0 + step + 1:c0 + step + 2])
                    elif step == d2:
                        if kfac:
                            nc.vector.tensor_tensor(acc[:], acc[:], K[:], ALU.mult)
                        done.add(m)
            # ---- PE: logits = sum_m <P_m, R_m>, m descending ----
            psL = psum.tile([128, V], F32, tag="psL", name=f"psL{r}")
            mm = [(m, ec) for m in range(NPOLY - 1, -1, -1) for ec in range(4)]
            if "mm" in skip:
                mm = mm[:1]
            for i, (m, ec) in enumerate(mm):
                nc.tensor.matmul(
                    psL[:], P[m][:, ec * 128:(ec + 1) * 128],
                    R[m][:, ec * V:(ec + 1) * V],
                    start=(i == 0), stop=False)
            # mask bias via rank-1 (ones128 x bvrow); the uniform 1+bl bias
            # cancels in log_softmax and is dropped entirely
            nc.tensor.matmul(psL[:], ones128, bvrow, start=False, stop=True)

            # ---- softmax tail (exp reads the biased logits in PSUM) ----
            ex = const.tile([128, V], F32, tag="ex", name=f"ex{r}")
            sm = const.tile([128, 1], F32, tag="sm", name=f"sm{r}")
            lsm = const.tile([128, 1], F32, tag="lsm", name=f"lsm{r}")
            ov = const.tile([128, V + 1], F32, tag="ov", name=f"ov{r}")
            nc.scalar.activation(ex[:], psL[:], AF.Exp, accum_out=sm[:])
            nc.scalar.activation(lsm[:], sm[:], AF.Ln)
            nc.vector.tensor_scalar_sub(ov[:, 0:V], psL[:], lsm[:, 0:1])
            nc.vector.tensor_scalar(
                ov[:, V:V + 1], lsm[:], -1.0, NEG,
                op0=ALU.mult, op1=ALU.add)
            # out rides the ACT queue: keeps the SP/Pool queues free so the
            # next repeat's loads aren't serialized behind this repeat's tail
            nc.scalar.dma_start(p_out[:], ov[:])

    nc.compile()
    return nc


def _fit_poly(X, n=NPOLY):
    """Least-squares odd-poly fit of tanh on [-X, X] at Chebyshev nodes."""
    x = X * np.cos(np.linspace(0, np.pi, 4001))
    pows = np.arange(1, n + 1, 2)
    M = x[:, None] ** pows[None, :]
    c, *_ = np.linalg.lstsq(M, np.tanh(x), rcond=None)
    cj = np.zeros(n + 1)
    cj[pows] = c
    return cj


def _prep(queries, keys, values, mask, Wc, bc, Wl, bl):
    """Host-side sharding: returns (V, in_maps, idx_valid, idx_masked)."""
    mask = np.asarray(mask)
    idx_v = [np.nonzero(mask[b])[0] for b in range(B)]
    idx_m = [np.nonzero(mask[b] == 0)[0] for b in range(B)]
    maxv = max(len(ix) for ix in idx_v)
    V = max(136, -(-maxv // 8) * 8)

    bf = ml_dtypes.bfloat16
    q_np = np.asarray(queries, np.float32)
    k_np = np.asarray(keys, np.float32)
    Wc_np = np.asarray(Wc, np.float32)
    bc_np = np.asarray(bc, np.float32)
    Wl_np = np.asarray(Wl, np.float32)[0]
    blv = float(np.asarray(bl, np.float32)[0])

    # poly fit on the exact realized range of A + K (computed on host; the
    # projections are cheap in fp32 BLAS and only run on the correctness path)
    qp = np.einsum('bhqd,ed->bqe', q_np, Wc_np[:, :D], optimize=True)
    kp = np.einsum('bhkd,ed->bke', k_np, Wc_np[:, D:], optimize=True) + bc_np
    xmax = max(float((qp[b].max(0) + kp[b].max(0)).max()) for b in range(B))
    xmin = min(float((qp[b].min(0) + kp[b].min(0)).min()) for b in range(B))
    X = max(abs(xmax), abs(xmin)) * 1.02
    cj = _fit_poly(X)
    coefs = np.asarray(_chain_coeffs(cj), np.float32)
    ncoef = len(coefs)

    wct_full = Wc_np.T.astype(bf)       # [2D, D]
    wct = np.empty((4, 128, 1024), bf)
    for ec in range(4):
        for dc in range(4):
            wct[ec, :, dc * 128:(dc + 1) * 128] = \
                wct_full[dc * 128:(dc + 1) * 128, ec * 128:(ec + 1) * 128]
            wct[ec, :, 512 + dc * 128:512 + (dc + 1) * 128] = \
                wct_full[D + dc * 128:D + (dc + 1) * 128, ec * 128:(ec + 1) * 128]
    wlp = Wl_np.reshape(4, 128).T
    p0 = np.repeat(wlp.T.astype(bf)[:, :, None], 128, axis=2) \
        .transpose(1, 0, 2).reshape(128, 512)
    aux = np.empty((128, 4 + ncoef), np.float32)
    aux[:, 0:4] = wlp
    aux[:, 4:] = coefs[None, :]
    aux = np.ascontiguousarray(aux)

    in_maps = []
    for c in range(NCORES):
        b, qh = c // 2, c % 2
        qt_d = q_np[b, 0, qh * LQL:(qh + 1) * LQL, :].T.astype(bf)   # [D, LQL]
        qt = qt_d.reshape(4, 128, LQL).transpose(1, 0, 2).reshape(128, 512)
        ktc = np.zeros((D, V), bf)
        ktc[:, :len(idx_v[b])] = k_np[b, 0, idx_v[b], :].T.astype(bf)
        kt = ktc.reshape(4, 128, V).transpose(1, 0, 2).reshape(128, 4 * V)
        # partition-0 rows for the rank-1 bias matmuls:
        #   [ones128 | bc chunks x4 | onesV | bvrow]
        zrow = np.zeros((128, 640 + 2 * V), bf)
        zrow[0, 0:128] = 1.0
        zrow[0, 128:640] = bc_np.astype(bf)
        zrow[0, 640:640 + V] = 1.0
        bvrow = np.zeros(V, np.float32)
        bvrow[len(idx_v[b]):] = NEG
        zrow[0, 640 + V:640 + 2 * V] = bvrow.astype(bf)
        qkp = np.concatenate([qt, p0, kt, zrow], axis=1)
        in_maps.append({
            "qkp": np.ascontiguousarray(qkp), "wct": wct, "aux": aux,
        })
    return V, in_maps, idx_v, idx_m


def kernel(queries, keys, values, mask, Wc, bc, Wl, bl):
    V, in_maps, idx_v, idx_m = _prep(queries, keys, values, mask, Wc, bc, Wl, bl)
    if V not in _nc_cache:
        _nc_cache[V] = _build(V)
    nc = _nc_cache[V]
    res = run_bass_kernel_spmd(nc, in_maps, core_ids=list(range(NCORES))).results

    full = np.empty((B, Lq, Lkv), np.float32)
    for c in range(NCORES):
        b, qh = c // 2, c % 2
        o = np.asarray(res[c]["out"], np.float32)      # [128, V+1]
        nv = len(idx_v[b])
        blk = full[b, qh * LQL:(qh + 1) * LQL]          # [128, Lkv]
        blk[:, idx_v[b]] = o[:, :nv]
        blk[:, idx_m[b]] = o[:, V:V + 1]
    return full
